# revision 15
# baseline (speedup 1.0000x reference)
"""Trainium2 Bass kernel for nn_Attn_55611236548746.

Attention pooling:
    energies[b,t] = enc[b,t,:]@w_e + hid_flat[b,:]@w_h + bias
    p = renorm(mask * softmax(energies * mask))
    out[b,:]     = sum_t p[b,t] * enc[b,t,:]

Sharding: data-parallel over B (32 batches -> 4 per core on 8 cores);
attn weights replicated.

Per-core design (memory regime):
  - enc converted to fp16 on the host: the kernel reads 16MB/core
    instead of 32MB and every engine gets 2-byte fast paths.
  - encoder tiles (128t x 16j x 1024e) fp16 stream via HWDGE (nc.sync),
    half-batch 2MB chunks, triple-buffered.
  - energies em[t] = enc_tile @ w_e, split over two engine paths per
    tile (HW-probed costs):
      A (6 tiles): DVE fused scalar_tensor_tensor+accum   (~1306ns, 1x)
      B (10 tiles): DVE tensor_tensor mult (~684ns, 2x) then ScalarE
         activation-Copy with accum_out (~1409ns) does the row-sum.
    This pins DVE ~= ScalarE ~= 15us/batch (an accumulating DVE op is
    always 1x; plain TT is the only 2x op available).
  - Exact softmax algebra: softmax denominator cancels against the
    final renorm, so p_t = mask_t*exp(en_t + h) / sum(...) for binary
    masks; exp runs on ScalarE with the hidden scalar as bias.
  - softmax + pooling run per half-batch (8 tiles) so the PE gets a
    pooling burst every ~7.5us (stays HAM-warm) and the tail after the
    last energy op is only half a pool.
  - weighted pool: fp16 PE matmuls contracting over t (u column as
    lhsT), fp32 PSUM accumulate; final scale by 1/sum(u) on ScalarE.
"""

import numpy as np

N_CORES = 8
B, T, E = 32, 2048, 1024
LD, HD = 2, 1024          # hidden: (LD, B, HD)
DEC = LD * HD             # 2048 = flattened-hidden width
BP = B // N_CORES         # 4 batches per core
TB = T // 128             # 16 t-blocks of 128

# Per half-batch (8 tiles): 4 A-path tiles (DVE fused STT+accum,
# ~1270ns/tile) and 2 B-path PAIRS (DVE tensor_tensor mult over a
# [128, 2048] pair at 2x ~1224ns/pair, then one ScalarE activation-Copy
# accum per tile ~1870ns). Balances DVE ~15.7us vs ScalarE ~15.9us per
# batch; any accumulating DVE op is 1x, plain TT is the only 2x op.
# half 0: 4 A + 2 pairs; half 1: 2 A + 3 pairs  (per batch: 6 A / 5 P)
_HALF_TILES = [
    [("A", 0), ("P", 1), ("A", 3), ("A", 4), ("P", 5), ("A", 7)],
    [("P", 0), ("A", 2), ("P", 3), ("A", 5), ("P", 6)],
]

_nc_cache = {}


def _build(reps=1, body_mult=1, mode="full"):
    """reps>1 wraps the main loop in a hardware For_i for benchmarking;
    body_mult repeats the whole 4-batch body inside the loop.
    mode: full | dma (loads only) | dve (loads+energies) — bench variants."""
    from contextlib import ExitStack

    import concourse.bacc as bacc
    import concourse.tile as tile
    from concourse import mybir
    from concourse._compat import with_exitstack
    from concourse.alu_op_type import AluOpType

    f32 = mybir.dt.float32
    f16 = mybir.dt.float16
    MUL, ADD = AluOpType.mult, AluOpType.add
    EXP = mybir.ActivationFunctionType.Exp
    COPY = mybir.ActivationFunctionType.Copy
    IDENT = mybir.ActivationFunctionType.Identity

    nc = bacc.Bacc("TRN2", target_bir_lowering=False, debug=False,
                   num_devices=N_CORES)
    enc = nc.dram_tensor("enc", [BP, T, E], f16, kind="ExternalInput").ap()
    hid = nc.dram_tensor("hid", [LD, BP, HD], f32, kind="ExternalInput").ap()
    msk = nc.dram_tensor("msk", [BP, T], f32, kind="ExternalInput").ap()
    wh = nc.dram_tensor("wh", [DEC], f32, kind="ExternalInput").ap()
    we = nc.dram_tensor("we", [E], f16, kind="ExternalInput").ap()
    bia = nc.dram_tensor("bia", [1], f32, kind="ExternalInput").ap()
    out = nc.dram_tensor("out", [BP, E], f32, kind="ExternalOutput").ap()

    @with_exitstack
    def body(ctx, tc):
        consts = ctx.enter_context(tc.tile_pool(name="consts", bufs=1))
        encp = ctx.enter_context(tc.tile_pool(name="encp", bufs=4))
        scrp = ctx.enter_context(tc.tile_pool(name="scrp", bufs=6))
        small = ctx.enter_context(tc.tile_pool(name="small", bufs=6))
        outp = ctx.enter_context(tc.tile_pool(name="outp", bufs=2))
        pso = ctx.enter_context(tc.tile_pool(name="pso", bufs=2, space="PSUM"))
        psh = ctx.enter_context(tc.tile_pool(name="psh", bufs=1, space="PSUM"))
        pst = ctx.enter_context(tc.tile_pool(name="pst", bufs=2, space="PSUM"))

        # ---- constants / per-core preamble ----
        # Tiny flat loads on the sync ring (few descriptors, fast), then
        # partition-broadcasts built on-chip with K=1 outer-product
        # matmuls — avoids the SWDGE small-descriptor storm that delayed
        # the first encoder chunks.
        we_sb = consts.tile([1, E], f16)
        nc.scalar.dma_start(out=we_sb, in_=we[None, :])
        bia_sb = consts.tile([1, 1], f32)
        nc.scalar.dma_start(out=bia_sb, in_=bia[None, :])
        wh_bc = consts.tile([BP, DEC], f32)
        nc.gpsimd.dma_start(out=wh_bc, in_=wh[None, :].to_broadcast([BP, DEC]))
        hid_sb = consts.tile([BP, LD, HD], f32)
        nc.gpsimd.dma_start(out=hid_sb, in_=hid.rearrange("l b e -> b l e"))
        mask_sb = consts.tile([128, BP, TB], f32)
        nc.gpsimd.dma_start(out=mask_sb, in_=msk.rearrange("b (p j) -> p b j", p=128))
        ones_col = consts.tile([128, 1], f32)
        nc.vector.memset(ones_col, 1.0)
        ones_row = consts.tile([1, 128], f32)
        nc.vector.memset(ones_row, 1.0)
        ones_row16 = consts.tile([1, 128], f16)
        nc.vector.memset(ones_row16, 1.0)
        # throwaway ACT output for the B-path row-sum (only accum_out used)
        junk = consts.tile([128, E], f16)

        # we_bc[p, e] = we[e] and b_bc128[p, 0] = bias via PE K=1
        # outer-product broadcasts (single PSUM tag: one bank, serialized)
        we_bc = consts.tile([128, E], f16)
        b_bc128 = consts.tile([128, 1], f32)
        for q in range(3):
            wps = psh.tile([128, 512], f32)
            if q < 2:
                sl = slice(512 * q, 512 * (q + 1))
                nc.tensor.matmul(wps, ones_row16, we_sb[:, sl],
                                 start=True, stop=True)
                nc.scalar.activation(out=we_bc[:, sl], in_=wps, func=COPY)
            else:
                nc.tensor.matmul(wps[:, 0:1], ones_row, bia_sb,
                                 start=True, stop=True)
                nc.scalar.activation(out=b_bc128, in_=wps[:, 0:1], func=COPY)
        we_bc2 = consts.tile([128, 2, E], f16)
        for q in range(2):
            nc.vector.tensor_copy(we_bc2[:, q, :], we_bc)

        # h[b] = hid_flat[b] . w_h, then broadcast to all partitions:
        # (4,1) column -> 32x32 DVE transpose -> (1,4) row -> k=1 outer-product
        # matmul with a ones row -> (128,4) in PSUM -> SBUF (+ bias via the
        # activation's per-partition bias input).
        h32 = consts.tile([32, 32], f32)
        nc.vector.memset(h32, 0.0)
        hscr = consts.tile([BP, DEC], f32)
        nc.vector.scalar_tensor_tensor(
            out=hscr, in0=hid_sb.rearrange("b l e -> b (l e)"), scalar=0.0,
            in1=wh_bc, op0=ADD, op1=MUL,
            accum_out=h32[0:BP, 0:1])
        h32t = consts.tile([32, 32], f32)
        nc.vector.transpose(out=h32t, in_=h32)
        h_ps = psh.tile([128, BP], f32)
        nc.tensor.matmul(h_ps, ones_row, h32t[0:1, 0:BP], start=True, stop=True)
        h_bc = consts.tile([128, BP], f32)
        nc.scalar.activation(out=h_bc, in_=h_ps, func=IDENT, bias=b_bc128,
                             scale=1.0)

        # ---- main loop over this core's batches ----
        def main_loop():
            for b in range(BP):
                # fp16 HBM -> fp16 SBUF via HWDGE, two 2MB chunks/batch.
                # Contiguous per-partition layout: t = 16*p + j -> each
                # partition reads one 16KB contiguous HBM run per chunk.
                enc_sb = encp.tile([128, TB, E], f16)
                encb = enc[b].rearrange("(p j) e -> p j e", p=128)
                widths = (2, 2, 4, 8) if b == 0 else (8, 8)
                j0 = 0
                for w_ in widths:
                    nc.sync.dma_start(
                        out=enc_sb[:, j0:j0 + w_, :],
                        in_=encb[:, j0:j0 + w_, :])
                    j0 += w_

                if mode == "dma":
                    sink = small.tile([1, 16], f16)
                    nc.vector.tensor_copy(sink, enc_sb[0:1, 0, 0:16])
                    continue

                en = small.tile([128, TB], f32)
                u = small.tile([128, TB], f16)
                us2 = small.tile([128, 2], f32)
                po = pso.tile([1, E], f32)

                for h in range(2):
                    # energies for this half's 8 tiles, two engine paths
                    for path, jj in _HALF_TILES[h]:
                        i = 8 * h + jj
                        if path == "A":
                            s = scrp.tile([128, E], f16)
                            nc.vector.scalar_tensor_tensor(
                                out=s, in0=enc_sb[:, i, :], scalar=0.0,
                                in1=we_bc, op0=ADD, op1=MUL,
                                accum_out=en[:, i:i + 1])
                        else:
                            # pair (i, i+1): one 2048-wide TT mult, then a
                            # ScalarE row-sum per tile
                            s = scrp.tile([128, 2, E], f16)
                            nc.vector.tensor_tensor(
                                out=s.rearrange("p a e -> p (a e)"),
                                in0=enc_sb[:, i:i + 2, :].rearrange(
                                    "p a e -> p (a e)"),
                                in1=we_bc2.rearrange("p a e -> p (a e)"), op=MUL)
                            for q in range(2):
                                nc.scalar.activation(
                                    out=junk, in_=s[:, q, :], func=COPY,
                                    accum_out=en[:, i + q:i + q + 1])

                    if mode == "dve":
                        continue

                    # u_half = mask * exp(en_half + h[b]); us2[:,h] = row-sums
                    sl8 = slice(8 * h, 8 * (h + 1))
                    u0 = small.tile([128, 8], f16)
                    nc.scalar.activation(out=u0, in_=en[:, sl8], func=EXP,
                                         bias=h_bc[:, b:b + 1], scale=1.0)
                    nc.vector.scalar_tensor_tensor(
                        out=u[:, sl8], in0=u0, scalar=0.0,
                        in1=mask_sb[:, b, sl8],
                        op0=ADD, op1=MUL, accum_out=us2[:, h:h + 1])

                    # weighted pool for this half's tiles:
                    # po[0,e] += sum u[t]*enc[t,e], fp16, fp32 PSUM accum
                    for eh in range(2):
                        sl = slice(eh * 512, (eh + 1) * 512)
                        for jj in range(8):
                            i = 8 * h + jj
                            nc.tensor.matmul(po[:, sl], u[:, i:i + 1],
                                             enc_sb[:, i, sl],
                                             start=(h == 0 and jj == 0),
                                             stop=(h == 1 and jj == 7),
                                             skip_group_check=True)

                if mode == "dve":
                    sink = small.tile([1, 16], f32)
                    nc.vector.tensor_copy(sink, en[0:1, :])
                    continue

                # total = sum_pt u ; rt = 1/total
                tot2 = pst.tile([1, 2], f32)
                nc.tensor.matmul(tot2, ones_col, us2, start=True, stop=True)
                rts = small.tile([1, 1], f32)
                nc.vector.tensor_reduce(out=rts, in_=tot2,
                                        axis=mybir.AxisListType.X, op=ADD)
                rt = small.tile([1, 1], f32)
                nc.vector.reciprocal(out=rt, in_=rts)

                ob = outp.tile([1, E], f32)
                nc.scalar.activation(out=ob, in_=po, func=COPY, scale=rt)
                nc.sync.dma_start(out=out[b], in_=ob)

        if reps == 1:
            for _ in range(body_mult):
                main_loop()
        else:
            with tc.For_i(0, reps, 1):
                for _ in range(body_mult):
                    main_loop()

    with tile.TileContext(nc) as tc:
        body(tc)
    nc.compile()
    return nc


def _get_nc(reps=1, body_mult=1, mode="full"):
    key = (reps, body_mult, mode)
    if key not in _nc_cache:
        _nc_cache[key] = _build(reps, body_mult, mode)
    return _nc_cache[key]


def _run(hidden, encoder_outputs, mask, attn_w, attn_b, trace=False,
         trace_kwargs=None, reps=1, body_mult=1, mode="full"):
    from concourse.bass_utils import run_bass_kernel_spmd

    nc = _get_nc(reps, body_mult, mode)
    enc16 = np.asarray(encoder_outputs, dtype=np.float16)
    we16 = np.ascontiguousarray(attn_w[DEC:]).astype(np.float16)
    wh32 = np.ascontiguousarray(attn_w[:DEC], dtype=np.float32)
    in_maps = []
    for i in range(N_CORES):
        lo = i * BP
        in_maps.append({
            "enc": np.ascontiguousarray(enc16[lo:lo + BP]),
            "hid": np.ascontiguousarray(hidden[:, lo:lo + BP, :]),
            "msk": np.ascontiguousarray(mask[lo:lo + BP]),
            "wh": wh32,
            "we": we16,
            "bia": np.ascontiguousarray(attn_b),
        })
    res = run_bass_kernel_spmd(nc, in_maps, list(range(N_CORES)),
                               trace=trace, **(trace_kwargs or {}))
    full = np.concatenate([res.results[i]["out"] for i in range(N_CORES)],
                          axis=0)
    return full, res


def kernel(hidden, encoder_outputs, mask, attn_w, attn_b):
    hidden = np.asarray(hidden, dtype=np.float32)
    encoder_outputs = np.asarray(encoder_outputs, dtype=np.float32)
    mask = np.asarray(mask, dtype=np.float32)
    attn_w = np.asarray(attn_w, dtype=np.float32)
    attn_b = np.asarray(attn_b, dtype=np.float32)
    full, _ = _run(hidden, encoder_outputs, mask, attn_w, attn_b)
    return full


# revision 16
# speedup vs baseline: 1.0973x; 1.0973x over previous
"""Trainium2 Bass kernel for nn_Attn_55611236548746.

Attention pooling:
    energies[b,t] = enc[b,t,:]@w_e + hid_flat[b,:]@w_h + bias
    p = renorm(mask * softmax(energies * mask))
    out[b,:]     = sum_t p[b,t] * enc[b,t,:]

Sharding: data-parallel over B (32 batches -> 4 per core on 8 cores);
attn weights replicated.

Per-core design (memory regime):
  - enc converted to fp16 on the host: the kernel reads 16MB/core
    instead of 32MB and every engine gets 2-byte fast paths.
  - encoder tiles (128t x 16j x 1024e) fp16 stream via HWDGE (nc.sync),
    half-batch 2MB chunks, triple-buffered.
  - energies em[t] = enc_tile @ w_e, split over two engine paths per
    tile (HW-probed costs):
      A (6 tiles): DVE fused scalar_tensor_tensor+accum   (~1306ns, 1x)
      B (10 tiles): DVE tensor_tensor mult (~684ns, 2x) then ScalarE
         activation-Copy with accum_out (~1409ns) does the row-sum.
    This pins DVE ~= ScalarE ~= 15us/batch (an accumulating DVE op is
    always 1x; plain TT is the only 2x op available).
  - Exact softmax algebra: softmax denominator cancels against the
    final renorm, so p_t = mask_t*exp(en_t + h) / sum(...) for binary
    masks; exp runs on ScalarE with the hidden scalar as bias.
  - softmax + pooling run per half-batch (8 tiles) so the PE gets a
    pooling burst every ~7.5us (stays HAM-warm) and the tail after the
    last energy op is only half a pool.
  - weighted pool: fp16 PE matmuls contracting over t (u column as
    lhsT), fp32 PSUM accumulate; final scale by 1/sum(u) on ScalarE.
"""

import numpy as np

N_CORES = 8
B, T, E = 32, 2048, 1024
LD, HD = 2, 1024          # hidden: (LD, B, HD)
DEC = LD * HD             # 2048 = flattened-hidden width
BP = B // N_CORES         # 4 batches per core
TB = T // 128             # 16 t-blocks of 128

# Per half-batch (8 tiles): 4 A-path tiles (DVE fused STT+accum,
# ~1270ns/tile) and 2 B-path PAIRS (DVE tensor_tensor mult over a
# [128, 2048] pair at 2x ~1224ns/pair, then one ScalarE activation-Copy
# accum per tile ~1870ns). Balances DVE ~15.7us vs ScalarE ~15.9us per
# batch; any accumulating DVE op is 1x, plain TT is the only 2x op.
# 4 A-tiles + 2 pairs per half (k=8/m=8 per batch). Pairs issue first so
# the trailing ScalarE accums overlap the A-tile STTs on DVE.
_HALF_TILES = [
    [("P", 0), ("P", 2), ("A", 4), ("A", 5), ("A", 6), ("A", 7)],
    [("P", 0), ("P", 2), ("A", 4), ("A", 5), ("A", 6), ("A", 7)],
]

_nc_cache = {}


def _build(reps=1, body_mult=1, mode="full"):
    """reps>1 wraps the main loop in a hardware For_i for benchmarking;
    body_mult repeats the whole 4-batch body inside the loop.
    mode: full | dma (loads only) | dve (loads+energies) — bench variants."""
    from contextlib import ExitStack

    import concourse.bacc as bacc
    import concourse.tile as tile
    from concourse import mybir
    from concourse._compat import with_exitstack
    from concourse.alu_op_type import AluOpType

    f32 = mybir.dt.float32
    f16 = mybir.dt.float16
    MUL, ADD = AluOpType.mult, AluOpType.add
    EXP = mybir.ActivationFunctionType.Exp
    COPY = mybir.ActivationFunctionType.Copy
    IDENT = mybir.ActivationFunctionType.Identity

    nc = bacc.Bacc("TRN2", target_bir_lowering=False, debug=False,
                   num_devices=N_CORES)
    enc = nc.dram_tensor("enc", [BP, T, E], f16, kind="ExternalInput").ap()
    hid = nc.dram_tensor("hid", [LD, BP, HD], f32, kind="ExternalInput").ap()
    msk = nc.dram_tensor("msk", [BP, T], f32, kind="ExternalInput").ap()
    wh = nc.dram_tensor("wh", [DEC], f32, kind="ExternalInput").ap()
    we = nc.dram_tensor("we", [E], f16, kind="ExternalInput").ap()
    bia = nc.dram_tensor("bia", [1], f32, kind="ExternalInput").ap()
    out = nc.dram_tensor("out", [BP, E], f32, kind="ExternalOutput").ap()

    @with_exitstack
    def body(ctx, tc):
        consts = ctx.enter_context(tc.tile_pool(name="consts", bufs=1))
        encp = ctx.enter_context(tc.tile_pool(name="encp", bufs=4))
        scrp = ctx.enter_context(tc.tile_pool(name="scrp", bufs=6))
        small = ctx.enter_context(tc.tile_pool(name="small", bufs=6))
        outp = ctx.enter_context(tc.tile_pool(name="outp", bufs=2))
        pso = ctx.enter_context(tc.tile_pool(name="pso", bufs=2, space="PSUM"))
        psh = ctx.enter_context(tc.tile_pool(name="psh", bufs=1, space="PSUM"))
        pst = ctx.enter_context(tc.tile_pool(name="pst", bufs=2, space="PSUM"))

        # ---- constants / per-core preamble ----
        # Tiny flat loads on the sync ring (few descriptors, fast), then
        # partition-broadcasts built on-chip with K=1 outer-product
        # matmuls — avoids the SWDGE small-descriptor storm that delayed
        # the first encoder chunks.
        we_sb = consts.tile([1, E], f16)
        nc.scalar.dma_start(out=we_sb, in_=we[None, :])
        bia_sb = consts.tile([1, 1], f32)
        nc.scalar.dma_start(out=bia_sb, in_=bia[None, :])
        wh_bc = consts.tile([BP, DEC], f32)
        nc.gpsimd.dma_start(out=wh_bc, in_=wh[None, :].to_broadcast([BP, DEC]))
        hid_sb = consts.tile([BP, LD, HD], f32)
        nc.gpsimd.dma_start(out=hid_sb, in_=hid.rearrange("l b e -> b l e"))
        mask_sb = consts.tile([128, BP, TB], f32)
        nc.gpsimd.dma_start(out=mask_sb, in_=msk.rearrange("b (p j) -> p b j", p=128))
        ones_col = consts.tile([128, 1], f32)
        nc.vector.memset(ones_col, 1.0)
        ones_row = consts.tile([1, 128], f32)
        nc.vector.memset(ones_row, 1.0)
        ones_row16 = consts.tile([1, 128], f16)
        nc.vector.memset(ones_row16, 1.0)
        # throwaway ACT output for the B-path row-sum (only accum_out used)
        junk = consts.tile([128, E], f16)

        # we_bc[p, e] = we[e] and b_bc128[p, 0] = bias via PE K=1
        # outer-product broadcasts (single PSUM tag: one bank, serialized)
        we_bc = consts.tile([128, E], f16)
        b_bc128 = consts.tile([128, 1], f32)
        for q in range(3):
            wps = psh.tile([128, 512], f32)
            if q < 2:
                sl = slice(512 * q, 512 * (q + 1))
                nc.tensor.matmul(wps, ones_row16, we_sb[:, sl],
                                 start=True, stop=True)
                nc.scalar.activation(out=we_bc[:, sl], in_=wps, func=COPY)
            else:
                nc.tensor.matmul(wps[:, 0:1], ones_row, bia_sb,
                                 start=True, stop=True)
                nc.scalar.activation(out=b_bc128, in_=wps[:, 0:1], func=COPY)
        we_bc2 = consts.tile([128, 2, E], f16)
        for q in range(2):
            nc.vector.tensor_copy(we_bc2[:, q, :], we_bc)

        # h[b] = hid_flat[b] . w_h, then broadcast to all partitions:
        # (4,1) column -> 32x32 DVE transpose -> (1,4) row -> k=1 outer-product
        # matmul with a ones row -> (128,4) in PSUM -> SBUF (+ bias via the
        # activation's per-partition bias input).
        h32 = consts.tile([32, 32], f32)
        nc.vector.memset(h32, 0.0)
        hscr = consts.tile([BP, DEC], f32)
        nc.vector.scalar_tensor_tensor(
            out=hscr, in0=hid_sb.rearrange("b l e -> b (l e)"), scalar=0.0,
            in1=wh_bc, op0=ADD, op1=MUL,
            accum_out=h32[0:BP, 0:1])
        h32t = consts.tile([32, 32], f32)
        nc.vector.transpose(out=h32t, in_=h32)
        h_ps = psh.tile([128, BP], f32)
        nc.tensor.matmul(h_ps, ones_row, h32t[0:1, 0:BP], start=True, stop=True)
        h_bc = consts.tile([128, BP], f32)
        nc.scalar.activation(out=h_bc, in_=h_ps, func=IDENT, bias=b_bc128,
                             scale=1.0)

        # ---- main loop over this core's batches ----
        def main_loop():
            for b in range(BP):
                # fp16 HBM -> fp16 SBUF via HWDGE, two 2MB chunks/batch.
                # Contiguous per-partition layout: t = 16*p + j -> each
                # partition reads one 16KB contiguous HBM run per chunk.
                enc_sb = encp.tile([128, TB, E], f16)
                encb = enc[b].rearrange("(p j) e -> p j e", p=128)
                widths = (2, 2, 4, 8) if b == 0 else (8, 8)
                j0 = 0
                for w_ in widths:
                    nc.sync.dma_start(
                        out=enc_sb[:, j0:j0 + w_, :],
                        in_=encb[:, j0:j0 + w_, :])
                    j0 += w_

                if mode == "dma":
                    sink = small.tile([1, 16], f16)
                    nc.vector.tensor_copy(sink, enc_sb[0:1, 0, 0:16])
                    continue

                en = small.tile([128, TB], f32)
                u = small.tile([128, TB], f16)
                us2 = small.tile([128, 2], f32)
                po = pso.tile([1, E], f32)

                for h in range(2):
                    # energies for this half's 8 tiles, two engine paths
                    for path, jj in _HALF_TILES[h]:
                        i = 8 * h + jj
                        if path == "A":
                            s = scrp.tile([128, E], f16)
                            nc.vector.scalar_tensor_tensor(
                                out=s, in0=enc_sb[:, i, :], scalar=0.0,
                                in1=we_bc, op0=ADD, op1=MUL,
                                accum_out=en[:, i:i + 1])
                        else:
                            # pair (i, i+1): one 2048-wide TT mult, then a
                            # ScalarE row-sum per tile
                            s = scrp.tile([128, 2, E], f16)
                            nc.vector.tensor_tensor(
                                out=s.rearrange("p a e -> p (a e)"),
                                in0=enc_sb[:, i:i + 2, :].rearrange(
                                    "p a e -> p (a e)"),
                                in1=we_bc2.rearrange("p a e -> p (a e)"), op=MUL)
                            for q in range(2):
                                nc.scalar.activation(
                                    out=junk, in_=s[:, q, :], func=COPY,
                                    accum_out=en[:, i + q:i + q + 1])

                    if mode == "dve":
                        continue

                    # u_half = mask * exp(en_half + h[b]); us2[:,h] = row-sums
                    sl8 = slice(8 * h, 8 * (h + 1))
                    u0 = small.tile([128, 8], f16)
                    nc.scalar.activation(out=u0, in_=en[:, sl8], func=EXP,
                                         bias=h_bc[:, b:b + 1], scale=1.0)
                    nc.vector.scalar_tensor_tensor(
                        out=u[:, sl8], in0=u0, scalar=0.0,
                        in1=mask_sb[:, b, sl8],
                        op0=ADD, op1=MUL, accum_out=us2[:, h:h + 1])

                    # weighted pool for this half's tiles:
                    # po[0,e] += sum u[t]*enc[t,e], fp16, fp32 PSUM accum
                    for eh in range(2):
                        sl = slice(eh * 512, (eh + 1) * 512)
                        for jj in range(8):
                            i = 8 * h + jj
                            nc.tensor.matmul(po[:, sl], u[:, i:i + 1],
                                             enc_sb[:, i, sl],
                                             start=(h == 0 and jj == 0),
                                             stop=(h == 1 and jj == 7),
                                             skip_group_check=True)

                if mode == "dve":
                    sink = small.tile([1, 16], f32)
                    nc.vector.tensor_copy(sink, en[0:1, :])
                    continue

                # total = sum_pt u ; rt = 1/total
                tot2 = pst.tile([1, 2], f32)
                nc.tensor.matmul(tot2, ones_col, us2, start=True, stop=True)
                rts = small.tile([1, 1], f32)
                nc.vector.tensor_reduce(out=rts, in_=tot2,
                                        axis=mybir.AxisListType.X, op=ADD)
                rt = small.tile([1, 1], f32)
                nc.vector.reciprocal(out=rt, in_=rts)

                ob = outp.tile([1, E], f32)
                nc.scalar.activation(out=ob, in_=po, func=COPY, scale=rt)
                nc.sync.dma_start(out=out[b], in_=ob)

        if reps == 1:
            for _ in range(body_mult):
                main_loop()
        else:
            with tc.For_i(0, reps, 1):
                for _ in range(body_mult):
                    main_loop()

    with tile.TileContext(nc) as tc:
        body(tc)
    nc.compile()
    return nc


def _get_nc(reps=1, body_mult=1, mode="full"):
    key = (reps, body_mult, mode)
    if key not in _nc_cache:
        _nc_cache[key] = _build(reps, body_mult, mode)
    return _nc_cache[key]


def _run(hidden, encoder_outputs, mask, attn_w, attn_b, trace=False,
         trace_kwargs=None, reps=1, body_mult=1, mode="full"):
    from concourse.bass_utils import run_bass_kernel_spmd

    nc = _get_nc(reps, body_mult, mode)
    enc16 = np.asarray(encoder_outputs, dtype=np.float16)
    we16 = np.ascontiguousarray(attn_w[DEC:]).astype(np.float16)
    wh32 = np.ascontiguousarray(attn_w[:DEC], dtype=np.float32)
    in_maps = []
    for i in range(N_CORES):
        lo = i * BP
        in_maps.append({
            "enc": np.ascontiguousarray(enc16[lo:lo + BP]),
            "hid": np.ascontiguousarray(hidden[:, lo:lo + BP, :]),
            "msk": np.ascontiguousarray(mask[lo:lo + BP]),
            "wh": wh32,
            "we": we16,
            "bia": np.ascontiguousarray(attn_b),
        })
    res = run_bass_kernel_spmd(nc, in_maps, list(range(N_CORES)),
                               trace=trace, **(trace_kwargs or {}))
    full = np.concatenate([res.results[i]["out"] for i in range(N_CORES)],
                          axis=0)
    return full, res


def kernel(hidden, encoder_outputs, mask, attn_w, attn_b):
    hidden = np.asarray(hidden, dtype=np.float32)
    encoder_outputs = np.asarray(encoder_outputs, dtype=np.float32)
    mask = np.asarray(mask, dtype=np.float32)
    attn_w = np.asarray(attn_w, dtype=np.float32)
    attn_b = np.asarray(attn_b, dtype=np.float32)
    full, _ = _run(hidden, encoder_outputs, mask, attn_w, attn_b)
    return full
